# revision 31
# baseline (speedup 1.0000x reference)
"""Trainium2 Bass kernel for a dense transformer decoder layer.

Tensor-parallel over 8 NeuronCores: each core owns 4 q-heads, 1 kv-head and
a 1/8 slice of the FFN hidden dim. Host<->device traffic is minimized (the
axon tunnel is the wall-clock bottleneck): x is shipped once as an fp16
token-shard per core and AllGathered on device; cos/sin ship as their unique
[32/64, S] blocks and are expanded on device; all weights ship fp16; the
final residual+FFN output is combined with an on-device ReduceScatter so
each core returns only a [256, T] fp16 feature slice.

Layout convention: activations are kept transposed as [feature, token] so the
contraction dim of every matmul is already on SBUF partitions. q/k head dims
are de-interleaved (x0 block then x1 block) so RoPE acts on 32-row blocks.
"""
import numpy as np

import concourse.bass as bass
import concourse.bacc as bacc
import concourse.tile as tile
from concourse import mybir
from concourse.masks import make_identity

F32 = mybir.dt.float32
F32R = mybir.dt.float32r
F16 = mybir.dt.float16
U8 = mybir.dt.uint8
U16 = mybir.dt.uint16
AF = mybir.ActivationFunctionType
OP = mybir.AluOpType

N_CORES = 8
EPS = 1e-5


def _blob_segments(c):
    """(name, rows, cols) of each fp16 piece packed into the per-core blob."""
    E, S, T = c["E"], c["S"], c["T"]
    Fc = c["F"] // N_CORES
    TSH = T // N_CORES
    return [("x_sh", E, TSH), ("wqkvT", E, 384), ("woT", 256, E),
            ("w1T", E, Fc), ("w3T", E, Fc), ("w2T", Fc, E),
            ("cos_u", 32, S), ("sinq_u", 64, S)]


def _cfg(S=2048, F=8192):
    B, E, HD = 2, 2048, 64
    T = B * S
    c = dict(B=B, S=S, E=E, F=F, HD=HD, T=T)
    c["KT_E"] = E // 128                 # k-tiles over E
    c["TCH"] = min(512, S)               # token chunk (== attention q chunk)
    c["NCH"] = T // c["TCH"]
    c["QC"] = min(512, S)                # attention q chunk
    c["NQC"] = S // c["QC"]
    c["KT_S"] = S // 128                 # k-tiles per batch (attention)
    c["Fc"] = F // N_CORES               # FFN rows per core
    return c


def build(cfg, collective=True):
    c = cfg
    E, T, TCH, NCH = c["E"], c["T"], c["TCH"], c["NCH"]
    KT_E, QC, NQC, KT_S = c["KT_E"], c["QC"], c["NQC"], c["KT_S"]
    B, S = c["B"], c["S"]
    Fc = c["Fc"]
    FM = Fc // 128                       # FFN hidden k/m tiles per core
    QKT = QC // 128                      # k-tiles inside one diagonal q chunk
    KT_C = TCH // 128                    # k-tiles per token chunk (attention V)
    TSH = T // N_CORES                   # token shard per core

    nc = bacc.Bacc(None, target_bir_lowering=False, debug=False)

    # ---- I/O: one packed blob per core, shipped as hi/lo byte planes (the
    # axon tunnel compresses; the hi-byte plane of fp16 weights is
    # low-entropy, so byte-planing transfers ~20% faster) ----
    segs = _blob_segments(c)
    blob_n = sum(p * f for _, p, f in segs)
    planes = nc.dram_tensor("planes", [2, blob_n], U8, kind="ExternalInput")
    out_pl = nc.dram_tensor("outp", [2, 256 * T], U8, kind="ExternalOutput")

    replica_groups = [list(range(N_CORES))]

    with tile.TileContext(nc) as tc:
        with (
            tc.tile_pool(name="dram", bufs=1, space="DRAM") as dram,
        ):
            xin = dram.tile([E, TSH], F16)
            xg = dram.tile([NCH, E, TSH], F16, tag="xg")
            o_bounce = dram.tile([NCH, E, TCH], F32)
            ff_part = dram.tile([E, T], F16)
            rs_out = dram.tile([256, T], F16)
            blob16 = dram.tile([blob_n], F16, tag="blob16")
            h2_shl = []
            for _tch in range(NCH):
                h2c_t = dram.tile([E, TCH], F32, addr_space="Shared",
                                  tag=f"h2sh{_tch}")
                h2_shl.append(h2c_t)

            # segment views into the decoded fp16 blob
            v = {}
            ofs = 0
            for name, p, f in segs:
                v[name] = blob16[ofs:ofs + p * f].rearrange("(p f) -> p f", p=p)
                ofs += p * f
            x_sh, wqkvT, woT = v["x_sh"], v["wqkvT"], v["woT"]
            w1T, w3T = v["w1T"], v["w3T"]
            cos_u, sinq_u = v["cos_u"], v["sinq_u"]
            w2_ofs = sum(p * f for n_, p, f in segs[:5])
            w2T3 = blob16[w2_ofs:w2_ofs + Fc * E].rearrange(
                "(kf p c) -> p kf c", p=128, c=E)        # [128, FM, E]

            # ---------- phase -1: byte-plane decode (u8 hi/lo -> f16) -------
            with (
                tc.tile_pool(name="decc", bufs=1) as decc,
                tc.tile_pool(name="dec", bufs=2) as dec,
            ):
                sh8 = decc.tile([128, 1], U16, tag="sh8")
                nc.vector.memset(sh8[:], 8)
                DK = 8192
                per_part = blob_n // 128
                j0 = 0
                while j0 < per_part:
                    k = min(DK, per_part - j0)
                    fsl = slice(j0 * 128, j0 * 128 + k * 128)
                    hi8 = dec.tile([128, DK], U8, tag="hi8")
                    lo8 = dec.tile([128, DK], U8, tag="lo8")
                    nc.sync.dma_start(
                        out=hi8[:, :k],
                        in_=planes[0, fsl].rearrange("(p f) -> p f", p=128))
                    nc.sync.dma_start(
                        out=lo8[:, :k],
                        in_=planes[1, fsl].rearrange("(p f) -> p f", p=128))
                    h16 = dec.tile([128, DK], U16, tag="h16")
                    l16 = dec.tile([128, DK], U16, tag="l16")
                    nc.vector.tensor_copy(out=h16[:, :k], in_=hi8[:, :k])
                    nc.vector.tensor_copy(out=l16[:, :k], in_=lo8[:, :k])
                    nc.vector.scalar_tensor_tensor(
                        out=h16[:, :k], in0=h16[:, :k], scalar=sh8[:],
                        in1=l16[:, :k],
                        op0=OP.logical_shift_left, op1=OP.bitwise_or)
                    nc.sync.dma_start(
                        out=blob16[fsl].rearrange("(p f) -> p f", p=128),
                        in_=h16[:, :k].bitcast(F16))
                    j0 += k

            # ---------- phase 0: AllGather the x token shards ----------
            nc.sync.dma_start(out=xin[:], in_=x_sh)
            if collective:
                nc.gpsimd.collective_compute(
                    "AllGather", OP.bypass, replica_groups=replica_groups,
                    ins=[xin[:].opt()], outs=[xg[:].opt()])
            else:
                for _tch in range(NCH):
                    nc.sync.dma_start(out=xg[_tch], in_=xin[:])

            gps_cm = tc.tile_pool(name="gps", bufs=2, space="PSUM")
            gps = gps_cm.__enter__()
            # manually-scoped pools with nested lifetimes: ao > qk
            ao_cm = tc.tile_pool(name="ao", bufs=1)          # .. oproj end
            ao_pool = ao_cm.__enter__()
            qk_cm = tc.tile_pool(name="qk", bufs=1)          # .. attention end
            qk = qk_cm.__enter__()

            # q/k tiles: written by qkv matmul epilogue, roped in place.
            qr0 = qk.tile([128, T], F32R, tag="qr0")   # q heads 0,1
            qr1 = qk.tile([128, T], F32R, tag="qr1")   # q heads 2,3
            kr = qk.tile([128, T], F32R, tag="kr")     # kv head x2
            vaug = qk.tile([128, B * KT_S, 65], F16, tag="vaug")

            # ---------- phase 1: qkv projection + rope + V transpose ----------
            with (
                tc.tile_pool(name="qkvw", bufs=1) as qkvw,
                tc.tile_pool(name="qkvx", bufs=2) as qkvx,
                tc.tile_pool(name="qkvs", bufs=2) as qkvs,
                tc.tile_pool(name="ropep", bufs=2) as ropep,
            ):
                wq_sb = qkvw.tile([128, KT_E, 384], F16, tag="wq")
                for kt in range(KT_E):
                    nc.sync.dma_start(out=wq_sb[:, kt, :],
                                      in_=wqkvT[kt * 128:(kt + 1) * 128, :])
                ident_f = qkvw.tile([64, 64], F32, tag="ident_f")
                make_identity(nc, ident_f[:])
                ident = qkvw.tile([64, 64], F32R, tag="ident")
                nc.vector.tensor_copy(out=ident[:], in_=ident_f[:])
                ones_f = qkvw.tile([128, 1], F32, tag="ones_f")
                nc.vector.memset(ones_f[:], 1.0)
                ones_r = qkvw.tile([128, 1], F32R, tag="ones_r")
                nc.vector.tensor_copy(out=ones_r[:], in_=ones_f[:])
                eps1q = qkvw.tile([1, 1], F32, tag="eps1q")
                nc.vector.memset(eps1q[:], EPS)

                for tch in range(NCH):
                    t0 = tch * TCH
                    tsl = slice(t0, t0 + TCH)
                    s0 = (tch % NQC) * TCH
                    ssl = slice(s0, s0 + TCH)
                    xch = qkvx.tile([128, KT_E, TCH], F16, tag="xch")
                    for kt in range(KT_E):
                        nc.sync.dma_start(
                            out=xch[:, kt, :],
                            in_=xg[tch, kt * 128:(kt + 1) * 128, :])
                    cos_sb = qkvs.tile([128, TCH], F16, tag="cos")
                    sin_sb = qkvs.tile([128, TCH], F16, tag="sin")
                    for j in range(4):
                        nc.sync.dma_start(out=cos_sb[j * 32:(j + 1) * 32, :],
                                          in_=cos_u[:, ssl])
                    for j in range(2):
                        nc.sync.dma_start(out=sin_sb[j * 64:(j + 1) * 64, :],
                                          in_=sinq_u[:, ssl])
                    # norm1 scales for this chunk (sum of squares over E via PE)
                    ps1c = gps.tile([1, TCH], F32, tag="n1")
                    for kt in range(KT_E):
                        sqx = qkvs.tile([128, TCH], F32R, tag="sqx")
                        nc.scalar.activation(out=sqx[:], in_=xch[:, kt, :],
                                             func=AF.Square)
                        nc.tensor.matmul(ps1c[:], ones_r[:], sqx[:],
                                         start=(kt == 0), stop=(kt == KT_E - 1))
                    st1 = qkvs.tile([1, TCH], F32, tag="st1")
                    nc.scalar.activation(out=st1[:], in_=ps1c[:], func=AF.Sqrt,
                                         scale=1.0 / E, bias=eps1q[:])
                    r01 = qkvs.tile([1, TCH], F32, tag="r01")
                    nc.vector.reciprocal(out=r01[:], in_=st1[:])
                    t11 = qkvs.tile([1, TCH], F32, tag="t11")
                    nc.vector.tensor_tensor(out=t11[:], in0=st1[:], in1=r01[:],
                                            op=OP.mult)
                    nc.vector.tensor_scalar(out=t11[:], in0=t11[:], scalar1=-1.0,
                                            scalar2=2.0, op0=OP.mult, op1=OP.add)
                    rr1 = qkvs.tile([1, TCH], F32, tag="rr1")
                    nc.vector.tensor_tensor(out=rr1[:], in0=r01[:], in1=t11[:],
                                            op=OP.mult)
                    s1b = qkvs.tile([128, TCH], F32, tag="s1b")
                    nc.gpsimd.partition_broadcast(s1b[:], rr1[:])
                    vT_c = qkvs.tile([64, TCH], F32R, tag="vT_c")
                    for dst, rows, m0 in ((qr0, 128, 0), (qr1, 128, 128),
                                          (kr, 64, 256), (vT_c, 64, 320)):
                        ps = gps.tile([128, TCH], F32, tag="mm")
                        for kt in range(KT_E):
                            nc.tensor.matmul(
                                ps[:rows, :],
                                wq_sb[:, kt, m0:m0 + rows],
                                xch[:, kt, :],
                                start=(kt == 0), stop=(kt == KT_E - 1))
                        if dst is vT_c:
                            nc.vector.tensor_tensor(
                                out=vT_c[:], in0=ps[:rows, :],
                                in1=s1b[:rows, :], op=OP.mult)
                        elif dst is kr:
                            # kv head duplicated into both 64-row halves (the
                            # PE needs lhsT/rhs on the same base partition)
                            for half in (0, 64):
                                nc.vector.tensor_tensor(
                                    out=kr[half:half + 64, tsl], in0=ps[:64, :],
                                    in1=s1b[:64, :], op=OP.mult)
                        else:
                            nc.vector.tensor_tensor(
                                out=dst[:, tsl], in0=ps[:],
                                in1=s1b[:], op=OP.mult)
                    # rope on this chunk (in place)
                    for qt in (qr0, qr1, kr):
                        swp = ropep.tile([128, TCH], F32, tag="swp")
                        for b0 in (0, 64):
                            nc.sync.dma_start(
                                out=swp[b0:b0 + 32, :],
                                in_=qt[b0 + 32:b0 + 64, tsl].bitcast(F32))
                            nc.sync.dma_start(
                                out=swp[b0 + 32:b0 + 64, :],
                                in_=qt[b0:b0 + 32, tsl].bitcast(F32))
                        tm = ropep.tile([128, TCH], F32, tag="tm")
                        nc.vector.tensor_tensor(out=tm[:], in0=qt[:, tsl].bitcast(F32),
                                                in1=cos_sb[:], op=OP.mult)
                        um = ropep.tile([128, TCH], F32, tag="um")
                        nc.vector.tensor_tensor(out=um[:], in0=swp[:],
                                                in1=sin_sb[:], op=OP.mult)
                        nc.vector.tensor_tensor(out=qt[:, tsl], in0=tm[:], in1=um[:],
                                                op=OP.add)
                    # V transpose for this chunk -> vaug (col 64 = ones)
                    for j in range(KT_C):
                        kt = tch * KT_C + j
                        pt = gps.tile([128, 64], F32R, tag="attv")
                        nc.tensor.transpose(pt[:], vT_c[:, j * 128:(j + 1) * 128],
                                            ident[:])
                        nc.vector.tensor_copy(out=vaug[:, kt, 0:64], in_=pt[:])
                        nc.vector.tensor_copy(out=vaug[:, kt, 64:65], in_=ones_f[:])

            # ---------- phase 2: attention -> o-proj -> chunked AR ----------
            aoT0 = ao_pool.tile([128, T], F16, tag="aoT0")
            aoT1 = ao_pool.tile([128, T], F16, tag="aoT1")
            wo_sb = ao_pool.tile([128, 2, E], F16, tag="wo_sb")
            for kt in range(2):
                nc.sync.dma_start(out=wo_sb[:, kt, :],
                                  in_=woT[kt * 128:(kt + 1) * 128, :])
            with (
                tc.tile_pool(name="att", bufs=1) as att,
                tc.tile_pool(name="atts", bufs=2) as atts,
                tc.tile_pool(name="opo", bufs=2) as opo,
            ):
                for b in range(B):
                    for qc in range(NQC):
                        qs = b * S + qc * QC
                        n_kb = qc * QKT + QKT
                        for (qtile, aoT) in [(qr0, aoT0), (qr1, aoT1)]:
                            expsA = att.tile([128, KT_S, QC], F16, tag="expsA")
                            expsB = att.tile([128, KT_S, QC], F16, tag="expsB")
                            exps = [expsA, expsB]
                            for kb in range(n_kb):
                                ksl = slice(b * S + kb * 128, b * S + kb * 128 + 128)
                                for h in range(2):
                                    ps = gps.tile([128, QC], F32, tag="sc")
                                    nc.tensor.matmul(
                                        ps[:],
                                        kr[h * 64:(h + 1) * 64, ksl],
                                        qtile[h * 64:(h + 1) * 64, qs:qs + QC],
                                        start=True, stop=True)
                                    nc.scalar.activation(
                                        out=exps[h][:, kb, :], in_=ps[:], func=AF.Exp)
                                    j = kb - qc * QKT
                                    if j >= 0:
                                        nc.gpsimd.affine_select(
                                            out=exps[h][:, kb, :],
                                            in_=exps[h][:, kb, :],
                                            compare_op=OP.is_ge,
                                            fill=0.0, base=-128 * j,
                                            pattern=[[1, QC]], channel_multiplier=-1)
                            for h in range(2):
                                po = gps.tile([65, QC], F32, tag="attv")
                                for kb in range(n_kb):
                                    gkt = b * KT_S + kb
                                    nc.tensor.matmul(
                                        po[:], vaug[:, gkt, :], exps[h][:, kb, :],
                                        start=(kb == 0), stop=(kb == n_kb - 1))
                                # softmax denominators live in row 64
                                ssb = atts.tile([1, QC], F32, tag="ssb")
                                nc.vector.tensor_copy(out=ssb[:], in_=po[64:65, :])
                                r0 = atts.tile([1, QC], F32, tag="r0")
                                nc.vector.reciprocal(out=r0[:], in_=ssb[:])
                                t1 = atts.tile([1, QC], F32, tag="t1")
                                nc.vector.tensor_tensor(out=t1[:], in0=ssb[:],
                                                        in1=r0[:], op=OP.mult)
                                nc.vector.tensor_scalar(
                                    out=t1[:], in0=t1[:], scalar1=-1.0, scalar2=2.0,
                                    op0=OP.mult, op1=OP.add)
                                rr = atts.tile([1, QC], F32, tag="rr")
                                nc.vector.tensor_tensor(out=rr[:], in0=r0[:],
                                                        in1=t1[:], op=OP.mult)
                                rb = atts.tile([64, QC], F32, tag="rb")
                                nc.gpsimd.partition_broadcast(rb[:], rr[:])
                                nc.vector.tensor_tensor(
                                    out=aoT[h * 64:(h + 1) * 64, qs:qs + QC],
                                    in0=po[0:64, :], in1=rb[:], op=OP.mult)
                        # ---- o-proj + x/8 for this token chunk, then AR ----
                        tch = b * NQC + qc
                        t0 = tch * TCH
                        for em in range(KT_E):
                            ps = gps.tile([128, TCH], F32, tag="mm")
                            for kt, ao_t in ((0, aoT0), (1, aoT1)):
                                nc.tensor.matmul(
                                    ps[:], wo_sb[:, kt, em * 128:(em + 1) * 128],
                                    ao_t[:, t0:t0 + TCH],
                                    start=(kt == 0), stop=(kt == 1))
                            x_em = opo.tile([128, TCH], F16, tag="x_em")
                            nc.sync.dma_start(
                                out=x_em[:],
                                in_=xg[tch, em * 128:(em + 1) * 128, :])
                            ob = opo.tile([128, TCH], F32, tag="ob")
                            nc.vector.scalar_tensor_tensor(
                                out=ob[:], in0=x_em[:], scalar=1.0 / N_CORES,
                                in1=ps[:], op0=OP.mult, op1=OP.add)
                            nc.sync.dma_start(
                                out=o_bounce[tch, em * 128:(em + 1) * 128, :],
                                in_=ob[:])
                        if collective:
                            nc.gpsimd.collective_compute(
                                "AllReduce", OP.add, replica_groups=replica_groups,
                                ins=[o_bounce[tch].opt()], outs=[h2_shl[tch].opt()])
                        else:
                            nc.sync.dma_start(out=h2_shl[tch][:], in_=o_bounce[tch])
            qk_cm.__exit__(None, None, None)
            ao_cm.__exit__(None, None, None)

            # ---------- phase 3: norm2 + FFN (fused, single pass) ----------
            with (
                tc.tile_pool(name="ffc", bufs=1) as ffc,
                tc.tile_pool(name="ffh2", bufs=2) as ffh2,
                tc.tile_pool(name="ffg", bufs=1) as ffg,
                tc.tile_pool(name="ffk", bufs=2) as ffk,
                tc.tile_pool(name="ffs", bufs=2) as ffs,
                tc.tile_pool(name="ffhf", bufs=1) as ffhf,
                tc.tile_pool(name="ffo", bufs=3) as ffo,
            ):
                ones_fb = ffc.tile([128, 1], F32, tag="ones_fb")
                nc.vector.memset(ones_fb[:], 1.0)
                ones_sb = ffc.tile([128, 1], F32R, tag="ones")
                nc.vector.tensor_copy(out=ones_sb[:], in_=ones_fb[:])
                eps1 = ffc.tile([1, 1], F32, tag="eps1")
                nc.vector.memset(eps1[:], EPS)
                w1h = ffc.tile([128, KT_E, Fc], F16, tag="w1h")
                w3h = ffc.tile([128, KT_E, Fc], F16, tag="w3h")
                for kt in range(KT_E):
                    nc.sync.dma_start(out=w1h[:, kt, :],
                                      in_=w1T[kt * 128:(kt + 1) * 128, :])
                    nc.sync.dma_start(out=w3h[:, kt, :],
                                      in_=w3T[kt * 128:(kt + 1) * 128, :])
                for tch in range(NCH):
                    t0 = tch * TCH
                    h2a = ffh2.tile([128, KT_E, TCH], F32, tag="h2a")
                    for kt in range(KT_E):
                        nc.sync.dma_start(
                            out=h2a[:, kt, :],
                            in_=h2_shl[tch][kt * 128:(kt + 1) * 128, :])
                    ps = gps.tile([1, TCH], F32, tag="n1")
                    for kt in range(KT_E):
                        sqc = ffk.tile([128, TCH], F32R, tag="sqc")
                        nc.scalar.activation(out=sqc[:], in_=h2a[:, kt, :],
                                             func=AF.Square)
                        nc.tensor.matmul(ps[:], ones_sb[:], sqc[:],
                                         start=(kt == 0), stop=(kt == KT_E - 1))
                    st = ffs.tile([1, TCH], F32, tag="st")
                    nc.scalar.activation(out=st[:], in_=ps[:], func=AF.Sqrt,
                                         scale=1.0 / E, bias=eps1[:])
                    r0 = ffs.tile([1, TCH], F32, tag="r0")
                    nc.vector.reciprocal(out=r0[:], in_=st[:])
                    t1 = ffs.tile([1, TCH], F32, tag="t1")
                    nc.vector.tensor_tensor(out=t1[:], in0=st[:], in1=r0[:], op=OP.mult)
                    nc.vector.tensor_scalar(out=t1[:], in0=t1[:], scalar1=-1.0,
                                            scalar2=2.0, op0=OP.mult, op1=OP.add)
                    rr = ffs.tile([1, TCH], F32, tag="rr")
                    nc.vector.tensor_tensor(out=rr[:], in0=r0[:], in1=t1[:], op=OP.mult)
                    s2b = ffs.tile([128, TCH], F32, tag="s2b")
                    nc.gpsimd.partition_broadcast(s2b[:], rr[:])
                    gc = ffg.tile([128, KT_E, TCH], F16, tag="gc")
                    for kt in range(KT_E):
                        nc.vector.tensor_tensor(out=gc[:, kt, :], in0=h2a[:, kt, :],
                                                in1=s2b[:], op=OP.mult)
                    hff = ffhf.tile([128, FM, TCH], F16, tag="hff")
                    for fm in range(FM):
                        ps1 = gps.tile([128, TCH], F32, tag="sc")
                        for kt in range(KT_E):
                            nc.tensor.matmul(
                                ps1[:], w1h[:, kt, fm * 128:(fm + 1) * 128],
                                gc[:, kt, :],
                                start=(kt == 0), stop=(kt == KT_E - 1))
                        h1 = ffhf.tile([128, TCH], F32, tag="h1")
                        nc.scalar.activation(out=h1[:], in_=ps1[:], func=AF.Silu)
                        ps3 = gps.tile([128, TCH], F32, tag="attv")
                        for kt in range(KT_E):
                            nc.tensor.matmul(
                                ps3[:], w3h[:, kt, fm * 128:(fm + 1) * 128],
                                gc[:, kt, :],
                                start=(kt == 0), stop=(kt == KT_E - 1))
                        nc.vector.tensor_tensor(out=hff[:, fm, :], in0=h1[:],
                                                in1=ps3[:], op=OP.mult)
                    for em in range(KT_E):
                        w2_em = ffk.tile([128, FM, 128], F16, tag="w2_em")
                        nc.sync.dma_start(
                            out=w2_em[:],
                            in_=w2T3[:, :, em * 128:(em + 1) * 128])
                        psd = gps.tile([128, TCH], F32, tag="mm")
                        for kf in range(FM):
                            nc.tensor.matmul(
                                psd[:], w2_em[:, kf, :],
                                hff[:, kf, :],
                                start=(kf == 0), stop=(kf == FM - 1))
                        od = ffo.tile([128, TCH], F16, tag="od")
                        nc.vector.scalar_tensor_tensor(
                            out=od[:], in0=h2a[:, em, :], scalar=1.0 / N_CORES,
                            in1=psd[:], op0=OP.mult, op1=OP.add)
                        nc.sync.dma_start(
                            out=ff_part[em * 128:(em + 1) * 128, t0:t0 + TCH],
                            in_=od[:])
            gps_cm.__exit__(None, None, None)

            # ---------- phase 4: ReduceScatter -> per-core output slice ----
            if collective:
                nc.gpsimd.collective_compute(
                    "ReduceScatter", OP.add, replica_groups=replica_groups,
                    ins=[ff_part[:].opt()], outs=[rs_out[:].opt()])
            else:
                nc.sync.dma_start(out=rs_out[:], in_=ff_part[0:256, :])
            # byte-plane encode the output (u8 planes compress on the tunnel)
            with tc.tile_pool(name="enc", bufs=2) as enc:
                sh8e = enc.tile([128, 1], U16, tag="sh8e")
                nc.vector.memset(sh8e[:], 8)
                m255 = enc.tile([128, 1], U16, tag="m255")
                nc.vector.memset(m255[:], 255)
                for j in range(2):
                    osl = slice(j * 128 * T, (j + 1) * 128 * T)
                    u = enc.tile([128, T], F16, tag="u")
                    nc.sync.dma_start(out=u[:],
                                      in_=rs_out[j * 128:(j + 1) * 128, :])
                    hs = enc.tile([128, T], U16, tag="hs")
                    nc.vector.tensor_scalar(
                        out=hs[:], in0=u[:].bitcast(U16), scalar1=sh8e[:],
                        scalar2=None, op0=OP.logical_shift_right)
                    ls = enc.tile([128, T], U16, tag="ls")
                    nc.vector.tensor_scalar(
                        out=ls[:], in0=u[:].bitcast(U16), scalar1=m255[:],
                        scalar2=None, op0=OP.bitwise_and)
                    h8 = enc.tile([128, T], U8, tag="h8")
                    l8 = enc.tile([128, T], U8, tag="l8")
                    nc.vector.tensor_copy(out=h8[:], in_=hs[:])
                    nc.vector.tensor_copy(out=l8[:], in_=ls[:])
                    nc.sync.dma_start(
                        out=out_pl[0, osl].rearrange("(p f) -> p f", p=128),
                        in_=h8[:])
                    nc.sync.dma_start(
                        out=out_pl[1, osl].rearrange("(p f) -> p f", p=128),
                        in_=l8[:])

    if not nc.is_finalized():
        nc.finalize()
    return nc


# ---------------------------------------------------------------------------
# host side
# ---------------------------------------------------------------------------

_DEINT = np.r_[np.arange(0, 64, 2), np.arange(1, 64, 2)]


def _prep_globals(x, freqs_cis, w_qkv, w_o, w1, w2, w3, attn_norm_w, ff_norm_w,
                  cfg):
    """Build the single packed global blob (per-core shards on axis 0)."""
    c = cfg
    B, S, E, F, T = c["B"], c["S"], c["E"], c["F"], c["T"]
    H, KH, HD = 32, 8, 64
    KV = KH * HD
    TSH = T // N_CORES
    Fc = F // N_CORES

    segs = _blob_segments(c)
    blob_n = sum(p * f for _, p, f in segs)
    blob_g = np.empty(N_CORES * blob_n, np.float16)
    bv = blob_g.reshape(N_CORES, blob_n)
    views = {}
    ofs = 0
    for name, p, f in segs:
        views[name] = bv[:, ofs:ofs + p * f].reshape(N_CORES, p, f)
        ofs += p * f

    x2 = np.asarray(x, dtype=np.float32).reshape(T, E)
    fc = np.asarray(freqs_cis, dtype=np.float32)       # [S, 32, 2]
    cos_u = fc[:, :, 0].T                              # [32, S]
    sin32 = fc[:, :, 1].T
    views["cos_u"][:] = cos_u[None]
    views["sinq_u"][:] = np.concatenate([-sin32, sin32], axis=0)[None]

    n1 = np.asarray(attn_norm_w, dtype=np.float32)
    n2 = np.asarray(ff_norm_w, dtype=np.float32)
    wq = np.asarray(w_qkv[:E], dtype=np.float32).reshape(H, HD, E)
    wk = np.asarray(w_qkv[E:E + KV], dtype=np.float32).reshape(KH, HD, E)
    wv = np.asarray(w_qkv[E + KV:], dtype=np.float32).reshape(KH, HD, E)
    w_o = np.asarray(w_o, dtype=np.float32)
    w1 = np.asarray(w1, dtype=np.float32)
    w3 = np.asarray(w3, dtype=np.float32)
    w2 = np.asarray(w2, dtype=np.float32)

    for core in range(N_CORES):
        views["x_sh"][core] = x2[core * TSH:(core + 1) * TSH, :].T
        rows = [wq[core * 4 + j][_DEINT] * 0.125 for j in range(4)]
        rows += [wk[core][_DEINT], wv[core]]
        wsh = np.concatenate(rows, axis=0) * n1[None, :]        # [384, E]
        views["wqkvT"][core] = wsh.T
        views["woT"][core] = w_o[:, core * 256:(core + 1) * 256].T
        fsl = slice(core * Fc, (core + 1) * Fc)
        views["w1T"][core] = (w1[fsl] * n2[None, :]).T
        views["w3T"][core] = (w3[fsl] * n2[None, :]).T
        views["w2T"][core] = w2[:, fsl].T
    # split into hi/lo byte planes: [8, 2, blob_n] u8, hi plane first
    pb = blob_g.view(np.uint8).reshape(N_CORES, blob_n, 2)
    planes_g = np.ascontiguousarray(pb[:, :, ::-1].transpose(0, 2, 1))
    return {"planes": planes_g.reshape(N_CORES * 2, blob_n)}


_BUILD_CACHE = {}


def _get_runner(cfg_key):
    """Compile-once runner: jit(shard_map(bass_exec)) over 8 cores, with the
    donated output buffer allocated on device (no host->device zero upload)."""
    if cfg_key in _BUILD_CACHE:
        return _BUILD_CACHE[cfg_key]

    import jax
    import jax.numpy as jnp
    from jax.sharding import Mesh, PartitionSpec, NamedSharding
    from jax.experimental.shard_map import shard_map
    from concourse import mybir as _mybir
    from concourse.bass2jax import (_bass_exec_p, partition_id_tensor,
                                    install_neuronx_cc_hook)

    install_neuronx_cc_hook()
    nc = build(_cfg(*cfg_key))

    in_names, out_names, out_avals = [], [], []
    partition_name = nc.partition_id_tensor.name if nc.partition_id_tensor else None
    for alloc in nc.m.functions[0].allocations:
        if not isinstance(alloc, _mybir.MemoryLocationSet):
            continue
        name = alloc.memorylocations[0].name
        if alloc.kind == "ExternalInput":
            if name != partition_name:
                in_names.append(name)
        elif alloc.kind == "ExternalOutput":
            out_names.append(name)
            out_avals.append(jax.core.ShapedArray(
                tuple(alloc.tensor_shape), _mybir.dt.np(alloc.dtype)))
    n_params = len(in_names)
    n_outs = len(out_names)
    all_names = list(in_names) + list(out_names)
    if partition_name is not None:
        all_names.append(partition_name)

    def _body(*args):
        operands = list(args)
        if partition_name is not None:
            operands.append(partition_id_tensor())
        outs = _bass_exec_p.bind(
            *operands,
            out_avals=tuple(out_avals),
            in_names=tuple(all_names),
            out_names=tuple(out_names),
            lowering_input_output_aliases=(),
            sim_require_finite=True,
            sim_require_nnan=True,
            nc=nc,
        )
        return tuple(outs)

    devices = jax.devices()[:N_CORES]
    mesh = Mesh(np.asarray(devices), ("core",))
    spec = NamedSharding(mesh, PartitionSpec("core"))
    donate = tuple(range(n_params, n_params + n_outs))
    sharded = jax.jit(
        shard_map(_body, mesh=mesh,
                  in_specs=(PartitionSpec("core"),) * (n_params + n_outs),
                  out_specs=(PartitionSpec("core"),) * n_outs,
                  check_rep=False),
        donate_argnums=donate, keep_unused=True)
    # donated output buffers (contents are fully overwritten by the kernel's
    # final DMA; np.zeros is calloc'd lazily so the host cost is nil)
    zero_shapes = [(N_CORES * a.shape[0], *a.shape[1:]) for a in out_avals]
    zero_dtypes = [a.dtype for a in out_avals]

    def zfn():
        return tuple(np.zeros(s, d) for s, d in zip(zero_shapes, zero_dtypes))

    runner = (sharded, zfn, in_names, out_names)
    _BUILD_CACHE[cfg_key] = runner
    return runner


def run(x, freqs_cis, w_qkv, w_o, w1, w2, w3, attn_norm_w, ff_norm_w,
        S=2048, F=8192):
    cfg = _cfg(S, F)
    g = _prep_globals(x, freqs_cis, w_qkv, w_o, w1, w2, w3,
                      attn_norm_w, ff_norm_w, cfg)
    sharded, zfn, in_names, out_names = _get_runner((S, F))
    zeros = zfn()
    out_arrs = sharded(*[g[n] for n in in_names], *zeros)
    op = np.asarray(out_arrs[0]).reshape(N_CORES, 2, 256 * cfg["T"])
    u16 = (op[:, 0].astype(np.uint16) << 8) | op[:, 1]
    outT = u16.view(np.float16).reshape(cfg["E"], cfg["T"])  # [E, T]
    out = outT.T.astype(np.float32).reshape(cfg["B"], S, cfg["E"])
    return out


def kernel(x, attention_mask, freqs_cis, w_qkv, w_o, w1, w2, w3,
           attn_norm_w, ff_norm_w):
    return run(x, freqs_cis, w_qkv, w_o, w1, w2, w3, attn_norm_w, ff_norm_w,
               S=2048, F=8192)
